# revision 5
# baseline (speedup 1.0000x reference)
"""Trainium2 Bass kernel for nn_MultiHeadAttention_79130477461654.

The reference einsum "nhqk,nhvd->nhqd" contracts k and v independently, so
out = (sum_k softmax(energy))*(sum_s v) = broadcast(sum_s v) since softmax
rows sum to 1.  With v = split_heads(x @ Wv) and the reference's direct
(n,h,q,d)->(n,s,e) reshape, the full output reduces to

    xs[n]    = sum_s x[n,s,:]                       (1024,)
    Z[n]     = xs[n] @ Wv                           (1024,)
    WoSum    = sum_m Wo[64m+d, :]  (d=0..63)        (64, 1024)
    T[n,h,:] = Z[n][64h:64h+64] @ WoSum + bo        (16, 1024)
    out[n, 64h+r, :] = T[n,h,:]   for r in 0..63

Sharding: data parallel over batch N=8, one batch per core; weights
replicated.  All arithmetic on-device; host only casts dtypes and
re-lays-out tensors.

v3 schedule (v1 51.2us -> v2 41.0us -> v3):
  - all bf16 streams: x 2 MiB, Wv 2 MiB, Wo 2 MiB in, out 2 MiB back.
  - x streams FIRST in natural [s, e] layout; the seq-sum runs on the
    TENSOR engine (xs = sum_chunks ones^T @ x_tile, PSUM-accumulated),
    not DVE (v2's serial 8x1.2us DVE reduces were the Z critical path).
    This also keeps PE HAM-warm from the first tile onward.
  - xs row -> column transpose (8 rank-1 matmuls) runs under the Wv
    stream; Z accumulation chases Wv tiles; the Z->ytx8 transpose dance
    runs under the Wo stream.
  - Wo streams LAST as two host-contiguous column halves, each as 4
    staggered 256 KB sub-tiles; the WoSum row-fold is fused into the T
    matmul as PSUM accumulation over row-blocks, so T halves chase the
    Wo stream on a warm PE and the out halves pipeline against it.
  - bias enters each psT half first as a K=1 matmul (ones x bo).
"""

import numpy as np

N, S, E, H, D = 8, 1024, 1024, 16, 64
NCORES = 8
P = 128  # partitions
NCHUNK = 8  # 1024 rows / 128


def build_nc():
    import concourse.bacc as bacc
    import concourse.mybir as mybir
    from concourse.tile import TileContext

    F32 = mybir.dt.float32
    BF16 = mybir.dt.bfloat16
    nc = bacc.Bacc("TRN2", target_bir_lowering=False, debug=False)

    xd = nc.declare_dram_parameter("x2", [S, E], BF16, isOutput=False)
    wvd = nc.declare_dram_parameter("Wv", [E, E], BF16, isOutput=False)
    # Wo re-laid-out on host as two contiguous column halves: [2048, 512]
    wod = nc.declare_dram_parameter("WoH", [2 * E, E // 2], BF16, isOutput=False)
    bod = nc.declare_dram_parameter("bo1", [1, E], BF16, isOutput=False)
    i2d = nc.declare_dram_parameter("I2", [D, P], BF16, isOutput=False)
    outd = nc.declare_dram_parameter("out", [S, E], BF16, isOutput=True)

    # two HWDGE queues: SP (sync) and ACT (scalar)
    dmae = [nc.sync, nc.scalar]
    Copy = mybir.ActivationFunctionType.Copy

    with TileContext(nc) as tc:
        with (
            tc.tile_pool(name="xin", bufs=NCHUNK) as xp,
            tc.tile_pool(name="wv", bufs=NCHUNK) as wvp,
            tc.tile_pool(name="wo", bufs=8) as wop,
            tc.tile_pool(name="small", bufs=1) as sp,
            tc.tile_pool(name="psX", bufs=1, space="PSUM") as psX,
            tc.tile_pool(name="psZ", bufs=1, space="PSUM") as psZ,
            tc.tile_pool(name="psS", bufs=1, space="PSUM") as psS,
            tc.tile_pool(name="psY", bufs=1, space="PSUM") as psY,
            tc.tile_pool(name="psT", bufs=1, space="PSUM") as psT,
        ):
            # tiny consts on the SWDGE queue so the HWDGE queues stream at once
            i2_sb = sp.tile([D, P], BF16)
            nc.gpsimd.dma_start(out=i2_sb[:], in_=i2d[:])
            bo_sb = sp.tile([1, E], BF16)
            nc.gpsimd.dma_start(out=bo_sb[:], in_=bod[:])
            ones18 = sp.tile([1, 8], BF16)
            nc.vector.memset(ones18[:], 1.0)
            ones128 = sp.tile([1, P], BF16)
            nc.vector.memset(ones128[:], 1.0)
            onescol = sp.tile([P, 1], BF16)
            nc.vector.memset(onescol[:], 1.0)

            # ---- input DMAs.  x first (8x256KB, natural [s, e] layout),
            #      then Wv (8x256KB), then Wo (8x256KB: column half A as 4
            #      staggered rb-pair sub-tiles, then half B).
            xr = xd.rearrange("(k p) e -> k p e", p=P)
            wr = wvd.rearrange("(k p) e -> k p e", p=P)
            wor = wod.rearrange("(t rb p) c -> t p rb c", rb=2, p=P)
            xts = [None] * NCHUNK
            wvt = [None] * NCHUNK
            wot = [None] * NCHUNK
            for k in range(NCHUNK):
                t = xp.tile([P, E], BF16, tag="xt")
                dmae[k % 2].dma_start(out=t[:], in_=xr[k])
                xts[k] = t
            for k in range(NCHUNK):
                t = wvp.tile([P, E], BF16, tag="wv")
                dmae[k % 2].dma_start(out=t[:], in_=wr[k])
                wvt[k] = t
            for i in range(NCHUNK):
                # i = 0..3 -> column half A (rb pairs), i = 4..7 -> half B
                t = wop.tile([P, 2 * (E // 2)], BF16, tag="wo")
                dmae[i % 2].dma_start(
                    out=t[:].rearrange("p (rb c) -> p rb c", rb=2), in_=wor[i]
                )
                wot[i] = t

            # ---- bias rows into the T PSUM groups first (consts arrive
            #      early): psT[m, e'] starts at bo[e'] via K=1 matmul
            ps_t = psT.tile([P, E], F32, tag="pst")
            for half in range(2):
                sl = slice(half * 512, half * 512 + 512)
                nc.tensor.matmul(
                    ps_t[:, sl],
                    ones128[0:1, :],
                    bo_sb[0:1, sl],
                    start=True,
                    stop=False,
                    skip_group_check=True,
                )

            # ---- xs row (1, 1024) = sum_s x[s, :] on the TENSOR engine:
            #      accumulate ones^T @ x_chunk over the 8 s-chunks.
            ps_xs = psX.tile([1, E], F32, tag="psx")
            for k in range(NCHUNK):
                for half in range(2):
                    sl = slice(half * 512, half * 512 + 512)
                    nc.tensor.matmul(
                        ps_xs[0:1, sl],
                        onescol[:, 0:1],
                        xts[k][:, sl],
                        start=(k == 0),
                        stop=(k == NCHUNK - 1),
                        skip_group_check=True,
                    )

            # ---- xs row -> xpb column [128, 8]: ACT bf16 copies (per half,
            #      pipelined with the rank-1 transposes), 8 rank-1 matmuls,
            #      DVE cast.
            xsrow = sp.tile([1, E], BF16)
            ps_xsT = psS.tile([P, NCHUNK], F32, tag="pss")
            for half in range(2):
                sl = slice(half * 512, half * 512 + 512)
                nc.scalar.activation(xsrow[0:1, sl], ps_xs[0:1, sl], func=Copy)
                for k in range(4 * half, 4 * half + 4):
                    nc.tensor.matmul(
                        ps_xsT[:, k : k + 1],
                        xsrow[0:1, k * P : (k + 1) * P],
                        ones18[0:1, 0:1],
                        start=True,
                        stop=True,
                        skip_group_check=True,
                    )
            xpb = sp.tile([P, NCHUNK], BF16)
            nc.vector.tensor_copy(xpb[:], ps_xsT[:])

            # ---- Z row (1, 1024) = xs @ Wv, accumulated chunk by chunk as
            #      the Wv stream delivers.
            ps_z = psZ.tile([1, E], F32, tag="psz")
            for k in range(NCHUNK):
                for half in range(2):
                    sl = slice(half * 512, half * 512 + 512)
                    nc.tensor.matmul(
                        ps_z[0:1, sl],
                        xpb[:, k : k + 1],
                        wvt[k][:, sl],
                        start=(k == 0),
                        stop=(k == NCHUNK - 1),
                        skip_group_check=True,
                    )

            # ---- Z transpose dance: srow (ACT bf16), 16 rank-1 matmuls
            #      ps_sft[d, 8h+rr] = Z[64h+d], dup matmul via I2, bf16 cast.
            srow = sp.tile([1, E], BF16)
            ps_sft = psS.tile([D, P], F32, tag="pss")
            for half in range(2):
                sl = slice(half * 512, half * 512 + 512)
                nc.scalar.activation(srow[0:1, sl], ps_z[0:1, sl], func=Copy)
                for h in range(8 * half, 8 * half + 8):
                    nc.tensor.matmul(
                        ps_sft[:, 8 * h : 8 * h + 8],
                        srow[0:1, h * D : (h + 1) * D],
                        ones18[0:1, :],
                        start=True,
                        stop=True,
                        skip_group_check=True,
                    )
            sft8 = sp.tile([D, P], BF16)
            nc.vector.tensor_copy(sft8[:], ps_sft[:])
            # dup matmul: ytx8[p, m] = sft8[p%64, m]  (I2[d,p]=1 iff d==p%64)
            ps_ytx = psY.tile([P, P], F32, tag="psy")
            nc.tensor.matmul(
                ps_ytx[:], i2_sb[:], sft8[:], start=True, stop=True,
                skip_group_check=True,
            )
            ytx8 = sp.tile([P, P], BF16)
            nc.vector.tensor_copy(ytx8[:], ps_ytx[:])

            # ---- T accumulation fused with the Wo row-fold, chasing the Wo
            #      sub-tiles: psT[:, half] += sum_rb ytx8 @ Wo[128rb+p, half];
            #      then bf16 copy and the broadcast store
            #      out[8m + r8, half] = tb8[m, half].
            tb8 = sp.tile([P, E], BF16)
            outr = outd.rearrange("(m r8) e -> m r8 e", r8=8)
            for half in range(2):
                sl = slice(half * 512, half * 512 + 512)
                for i in range(4):
                    wt = wot[4 * half + i]
                    for rb in range(2):
                        nc.tensor.matmul(
                            ps_t[:, sl],
                            ytx8[:],
                            wt[:, rb * 512 : rb * 512 + 512],
                            start=False,
                            stop=(i == 3 and rb == 1),
                            skip_group_check=True,
                        )
                nc.vector.tensor_copy(tb8[:, sl], ps_t[:, sl])
                dmae[half].dma_start(
                    out=outr[:, :, sl],
                    in_=tb8[:, None, sl].to_broadcast((P, 8, 512)),
                )

    nc.compile()
    return nc


_NC_CACHE = None


def make_in_maps(x, Wv, Wo, bo):
    import ml_dtypes

    BF = ml_dtypes.bfloat16
    x = np.asarray(x, dtype=np.float32)
    Wv = np.ascontiguousarray(np.asarray(Wv, dtype=np.float32).astype(BF))
    Wo = np.asarray(Wo, dtype=np.float32).astype(BF)
    WoH = np.ascontiguousarray(np.concatenate([Wo[:, :512], Wo[:, 512:]], axis=0))
    bo1 = np.asarray(bo, dtype=np.float32).astype(BF).reshape(1, E)
    I2 = np.zeros((D, P), dtype=BF)
    I2[np.arange(P) % D, np.arange(P)] = 1.0
    return [
        {
            "x2": np.ascontiguousarray(x[j].astype(BF)),
            "Wv": Wv,
            "WoH": WoH,
            "bo1": bo1,
            "I2": I2,
        }
        for j in range(NCORES)
    ]


def kernel(x, Wq=None, Wk=None, Wv=None, Wo=None, bo=None, **_unused):
    from concourse.bass_utils import run_bass_kernel_spmd

    global _NC_CACHE
    if _NC_CACHE is None:
        _NC_CACHE = build_nc()
    nc = _NC_CACHE

    in_maps = make_in_maps(x, Wv, Wo, bo)
    res = run_bass_kernel_spmd(nc, in_maps, core_ids=list(range(NCORES))).results
    return np.stack(
        [res[j]["out"].astype(np.float32) for j in range(NCORES)], axis=0
    )


# revision 8
# speedup vs baseline: 1.0870x; 1.0870x over previous
"""Trainium2 Bass kernel for nn_MultiHeadAttention_79130477461654.

The reference einsum "nhqk,nhvd->nhqd" contracts k and v independently, so
out = (sum_k softmax(energy))*(sum_s v) = broadcast(sum_s v) since softmax
rows sum to 1.  With v = split_heads(x @ Wv) and the reference's direct
(n,h,q,d)->(n,s,e) reshape, the full output reduces to

    xs[n]    = sum_s x[n,s,:]                       (1024,)
    Z[n]     = xs[n] @ Wv                           (1024,)
    WoSum    = sum_m Wo[64m+d, :]  (d=0..63)        (64, 1024)
    T[n,h,:] = Z[n][64h:64h+64] @ WoSum + bo        (16, 1024)
    out[n, 64h+r, :] = T[n,h,:]   for r in 0..63

Sharding: data parallel over batch N=8, one batch per core; weights
replicated.  All arithmetic on-device; host only casts dtypes and
re-lays-out tensors.

v4 schedule (v1 51.2us -> v2 41.0us -> v3 47.3 -> v4):
  - all-bf16 streams: x^T 2 MiB + Wv 2 MiB pair-interleaved first, Wo
    2 MiB last (column-half-major, 8 staggered 256 KB sub-tiles), out
    2 MiB back as two column-half broadcast DMAs.
  - per-chunk seq-reduces of x split DVE (even chunks) / GpSimd (odd
    chunks) so they chase the stream concurrently (v2 serialized 8x1.2us
    on DVE alone; v3's PE version overloaded the tensor engine).
  - a ~2-3us PE warmup filler (const matmuls gated on a late Wv tile)
    holds the HAM clock gate at 8/8 through the tail, so the transpose
    dance and T chain run at 2.4 GHz (v2 ran them at 1.2 GHz: HAM trace
    showed K=4/8 from 20.4us on).
  - WoSum row-fold fused into the T matmuls (PSUM accumulation over row
    blocks, chasing Wo sub-tiles); bias enters each psT half first as a
    K=1 matmul; tail copies split ACT/DVE.
"""

import numpy as np

N, S, E, H, D = 8, 1024, 1024, 16, 64
NCORES = 8
P = 128  # partitions
NCHUNK = 8  # 1024 rows / 128
NFILL = 18  # PE warmup matmuls


def build_nc():
    import concourse.bacc as bacc
    import concourse.mybir as mybir
    from concourse.tile import TileContext

    F32 = mybir.dt.float32
    BF16 = mybir.dt.bfloat16
    nc = bacc.Bacc("TRN2", target_bir_lowering=False, debug=False)

    xtd = nc.declare_dram_parameter("xT", [E, S], BF16, isOutput=False)
    wvd = nc.declare_dram_parameter("Wv", [E, E], BF16, isOutput=False)
    # Wo re-laid-out on host as two contiguous column halves: [2048, 512]
    wod = nc.declare_dram_parameter("WoH", [2 * E, E // 2], BF16, isOutput=False)
    bod = nc.declare_dram_parameter("bo1", [1, E], BF16, isOutput=False)
    i2d = nc.declare_dram_parameter("I2", [D, P], BF16, isOutput=False)
    outd = nc.declare_dram_parameter("out", [S, E], BF16, isOutput=True)

    # two HWDGE queues: SP (sync) and ACT (scalar)
    dmae = [nc.sync, nc.scalar]
    Copy = mybir.ActivationFunctionType.Copy

    with TileContext(nc) as tc:
        with (
            tc.tile_pool(name="xin", bufs=NCHUNK) as xp,
            tc.tile_pool(name="wv", bufs=NCHUNK) as wvp,
            tc.tile_pool(name="wo", bufs=NCHUNK) as wop,
            tc.tile_pool(name="small", bufs=1) as sp,
            tc.tile_pool(name="psZ", bufs=1, space="PSUM") as psZ,
            tc.tile_pool(name="psS", bufs=1, space="PSUM") as psS,
            tc.tile_pool(name="psY", bufs=1, space="PSUM") as psY,
            tc.tile_pool(name="psT", bufs=1, space="PSUM") as psT,
        ):
            # tiny consts on the SWDGE queue so the HWDGE queues stream at once
            i2_sb = sp.tile([D, P], BF16)
            nc.gpsimd.dma_start(out=i2_sb[:], in_=i2d[:])
            bo_sb = sp.tile([1, E], BF16)
            nc.gpsimd.dma_start(out=bo_sb[:], in_=bod[:])
            ones18 = sp.tile([1, 8], BF16)
            nc.vector.memset(ones18[:], 1.0)
            ones128 = sp.tile([1, P], BF16)
            nc.vector.memset(ones128[:], 1.0)
            onescol = sp.tile([P, 1], BF16)
            nc.vector.memset(onescol[:], 1.0)

            # ---- input DMAs: (x_k, Wv_k) pairs alternating rings so the
            #      reduces and Z accumulation chase the stream; Wo last.
            xr = xtd.rearrange("(k p) s -> k p s", p=P)
            wr = wvd.rearrange("(k p) e -> k p e", p=P)
            wor = wod.rearrange("(t rb p) c -> t p rb c", rb=2, p=P)
            xts = [None] * NCHUNK
            wvt = [None] * NCHUNK
            wot = [None] * NCHUNK
            for k in range(NCHUNK):
                t = xp.tile([P, S], BF16, tag="xt")
                dmae[k % 2].dma_start(out=t[:], in_=xr[k])
                xts[k] = t
                t = wvp.tile([P, E], BF16, tag="wv")
                dmae[k % 2].dma_start(out=t[:], in_=wr[k])
                wvt[k] = t
            for i in range(NCHUNK):
                # i = 0..3 -> column half A (rb-pair sub-tiles), 4..7 -> B
                t = wop.tile([P, 2 * (E // 2)], BF16, tag="wo")
                dmae[i % 2].dma_start(
                    out=t[:].rearrange("p (rb c) -> p rb c", rb=2), in_=wor[i]
                )
                wot[i] = t

            # ---- bias rows into the T PSUM groups first (consts arrive
            #      early): psT[m, e'] starts at bo[e'] via K=1 matmul
            ps_t = psT.tile([P, E], F32, tag="pst")
            for half in range(2):
                sl = slice(half * 512, half * 512 + 512)
                nc.tensor.matmul(
                    ps_t[:, sl],
                    ones128[0:1, :],
                    bo_sb[0:1, sl],
                    start=True,
                    stop=False,
                    skip_group_check=True,
                )

            # ---- per-chunk seq-reduce of x: even chunks on DVE
            #      (tensor_reduce), odd chunks on ACT (activation Copy with
            #      accum_out = free-dim sum), so they chase the stream
            #      concurrently.  Both accumulate fp32 internally.
            xpb = sp.tile([P, NCHUNK], BF16)
            xacc = sp.tile([P, NCHUNK], F32)
            scratch = sp.tile([P, S], BF16)
            with nc.allow_low_precision(
                reason="reduce accumulates fp32 internally; bf16 only on write"
            ):
                for k in range(NCHUNK):
                    if k % 2 == 0:
                        nc.vector.tensor_reduce(
                            xpb[:, k : k + 1],
                            xts[k][:],
                            axis=mybir.AxisListType.X,
                            op=mybir.AluOpType.add,
                        )
                    else:
                        nc.scalar.activation(
                            scratch[:], xts[k][:], func=Copy,
                            accum_out=xacc[:, k : k + 1],
                        )
                        nc.vector.tensor_copy(xpb[:, k : k + 1], xacc[:, k : k + 1])

            # ---- Z row (1, 1024) = xs @ Wv, chunk-accumulated, chasing the
            #      (x_k, Wv_k) pairs; PE warmup filler between pairs 5 and 6
            #      (gated on Wv_5) so HAM is at 8/8 for the tail.
            ps_z = psZ.tile([1, E], F32, tag="psz")
            ps_ytx = psY.tile([P, P], F32, tag="psy")
            for k in range(NCHUNK):
                for half in range(2):
                    sl = slice(half * 512, half * 512 + 512)
                    nc.tensor.matmul(
                        ps_z[0:1, sl],
                        xpb[:, k : k + 1],
                        wvt[k][:, sl],
                        start=(k == 0),
                        stop=(k == NCHUNK - 1),
                        skip_group_check=True,
                    )
                if k == 5:
                    for f in range(NFILL):
                        nc.tensor.matmul(
                            ps_ytx[0:1, :],
                            onescol[:, 0:1],
                            wvt[5][:, (f % 8) * P : (f % 8) * P + P],
                            start=True,
                            stop=True,
                            skip_group_check=True,
                        )

            # ---- Z transpose dance: srow (ACT bf16, per half), 16 rank-1
            #      matmuls ps_sft[d, 8h+rr] = Z[64h+d], dup matmul via I2.
            srow = sp.tile([1, E], BF16)
            ps_sft = psS.tile([D, P], F32, tag="pss")
            for half in range(2):
                sl = slice(half * 512, half * 512 + 512)
                nc.scalar.activation(srow[0:1, sl], ps_z[0:1, sl], func=Copy)
                for h in range(8 * half, 8 * half + 8):
                    nc.tensor.matmul(
                        ps_sft[:, 8 * h : 8 * h + 8],
                        srow[0:1, h * D : (h + 1) * D],
                        ones18[0:1, :],
                        start=True,
                        stop=True,
                        skip_group_check=True,
                    )
            sft8 = sp.tile([D, P], BF16)
            nc.scalar.activation(sft8[:], ps_sft[:], func=Copy)
            # dup matmul: ytx8[p, m] = sft8[p%64, m]  (I2[d,p]=1 iff d==p%64)
            nc.tensor.matmul(
                ps_ytx[:], i2_sb[:], sft8[:], start=True, stop=True,
                skip_group_check=True,
            )
            ytx8 = sp.tile([P, P], BF16)
            nc.vector.tensor_copy(ytx8[:], ps_ytx[:])

            # ---- T accumulation fused with the Wo row-fold, chasing the Wo
            #      sub-tiles; then bf16 copy (ACT for half A, DVE for B) and
            #      the broadcast store out[8m + r8, half] = tb8[m, half].
            tb8 = sp.tile([P, E], BF16)
            outr = outd.rearrange("(m r8) e -> m r8 e", r8=8)
            for half in range(2):
                sl = slice(half * 512, half * 512 + 512)
                for i in range(4):
                    wt = wot[4 * half + i]
                    for rb in range(2):
                        nc.tensor.matmul(
                            ps_t[:, sl],
                            ytx8[:],
                            wt[:, rb * 512 : rb * 512 + 512],
                            start=False,
                            stop=(i == 3 and rb == 1),
                            skip_group_check=True,
                        )
                if half == 0:
                    nc.scalar.activation(tb8[:, sl], ps_t[:, sl], func=Copy)
                else:
                    nc.vector.tensor_copy(tb8[:, sl], ps_t[:, sl])
                dmae[half].dma_start(
                    out=outr[:, :, sl],
                    in_=tb8[:, None, sl].to_broadcast((P, 8, 512)),
                )

    nc.compile()
    return nc


_NC_CACHE = None


def make_in_maps(x, Wv, Wo, bo):
    import ml_dtypes

    BF = ml_dtypes.bfloat16
    x = np.asarray(x, dtype=np.float32)
    Wv = np.ascontiguousarray(np.asarray(Wv, dtype=np.float32).astype(BF))
    Wo = np.asarray(Wo, dtype=np.float32).astype(BF)
    WoH = np.ascontiguousarray(np.concatenate([Wo[:, :512], Wo[:, 512:]], axis=0))
    bo1 = np.asarray(bo, dtype=np.float32).astype(BF).reshape(1, E)
    I2 = np.zeros((D, P), dtype=BF)
    I2[np.arange(P) % D, np.arange(P)] = 1.0
    return [
        {
            "xT": np.ascontiguousarray(x[j].T.astype(BF)),
            "Wv": Wv,
            "WoH": WoH,
            "bo1": bo1,
            "I2": I2,
        }
        for j in range(NCORES)
    ]


def kernel(x, Wq=None, Wk=None, Wv=None, Wo=None, bo=None, **_unused):
    from concourse.bass_utils import run_bass_kernel_spmd

    global _NC_CACHE
    if _NC_CACHE is None:
        _NC_CACHE = build_nc()
    nc = _NC_CACHE

    in_maps = make_in_maps(x, Wv, Wo, bo)
    res = run_bass_kernel_spmd(nc, in_maps, core_ids=list(range(NCORES))).results
    return np.stack(
        [res[j]["out"].astype(np.float32) for j in range(NCORES)], axis=0
    )
